# revision 10
# baseline (speedup 1.0000x reference)
"""Causal self-attention + cross-attention Trainium2 kernel (8 NeuronCores).

Sharding: head-parallel. 16 heads x 2 batches = 32 (b,h) pairs; core c owns
heads {2c, 2c+1} for both batches (its 128 channels of C=1024). Projections
are column-sliced per core; attention runs fully local per head; the output
projection is row-sliced and the 8 partial [B*T, C] outputs are summed on
the host (no device collectives).

Numerics: float32r matmuls (fp32 data rounded to 11-bit mantissa; products
are exact in fp32, PSUM accumulates fp32). Softmax without max-subtraction
(scores are bounded ~|8| for this problem's distributions), exp on ScalarE
with the 1/sqrt(D) scale folded in, scores computed transposed (ST[k,q]) so
no probability transpose is needed before AV.
"""
import sys

sys.path.insert(0, "/opt/trn_rl_repo")

import numpy as np

import concourse.bass as bass
import concourse.tile as tile
from concourse import bacc, mybir
from concourse.bass_utils import run_bass_kernel_spmd
from concourse.masks import make_identity
from concourse.tile import add_dep_helper

dt = mybir.dt

B, T, TC, C, CC, H, D = 2, 2048, 512, 1024, 512, 16, 64
NCORES = 8
CPC = 128          # channels per core = 2 heads * 64
NT = B * T         # 4096 tokens (batch-major)
NTC = B * TC       # 1024 cross tokens
KT_X = C // 128    # 8 contraction tiles over C
KT_C = CC // 128   # 4 contraction tiles over CC
NCH = NT // 512    # 8 token chunks
NCHC = NTC // 512  # 2 cross token chunks
QC_PER_B = T // 512  # 4 q-chunks per batch
KT_PER_B = T // 128  # 16 k-tiles per batch


def _round_fp32r(x):
    """Round fp32 array to fp32r (11-bit mantissa, RNE) on the host."""
    u = np.ascontiguousarray(x, np.float32).view(np.uint32).copy()
    u += 0x7FF + ((u >> 12) & 1)
    u &= 0xFFFFF000
    return u.view(np.float32)


def _build(zero_bias=False):
    f32, f32r = dt.float32, dt.float32r
    nc = bacc.Bacc("TRN2", target_bir_lowering=False, debug=False,
                   enable_asserts=True, num_devices=NCORES)

    xT = nc.dram_tensor("xT", [C, NT], f32r, kind="ExternalInput").ap()
    cT = nc.dram_tensor("cT", [CC, NTC], f32r, kind="ExternalInput").ap()
    wq = nc.dram_tensor("wq", [C, CPC], f32r, kind="ExternalInput").ap()
    wk = nc.dram_tensor("wk", [C, CPC], f32r, kind="ExternalInput").ap()
    wv = nc.dram_tensor("wv", [C, CPC], f32r, kind="ExternalInput").ap()
    wcq = nc.dram_tensor("wcq", [C, CPC], f32r, kind="ExternalInput").ap()
    wck = nc.dram_tensor("wck", [CC, CPC], f32r, kind="ExternalInput").ap()
    wcv = nc.dram_tensor("wcv", [CC, CPC], f32r, kind="ExternalInput").ap()
    wp = nc.dram_tensor("wp", [CPC, C], f32r, kind="ExternalInput").ap()
    bias6 = nc.dram_tensor("bias6", [CPC, 6], f32, kind="ExternalInput").ap()
    maskd = nc.dram_tensor("mask", [128, 128], dt.float16, kind="ExternalInput").ap()
    out = nc.dram_tensor("out", [NT, C], f32, kind="ExternalOutput").ap()

    Exp = mybir.ActivationFunctionType.Exp
    SCALE = 0.125  # 1/sqrt(D)

    with tile.TileContext(nc) as tc:
        from contextlib import ExitStack
        with ExitStack() as es:
            persist = es.enter_context(tc.tile_pool(name="persist", bufs=1))
            qT_t = persist.tile([128, NT], f32r, tag="qT")
            kT_t = persist.tile([128, NT], f32r, tag="kT")
            qcT_t = persist.tile([128, NT], f32r, tag="qcT")
            kcT_t = persist.tile([128, NTC], f32r, tag="kcT")
            vn_t = persist.tile([128, NT], dt.float16, tag="vn")     # v natural, col=tile*128+ch
            vcn_t = persist.tile([128, NTC], dt.float16, tag="vcn")
            yT2_t = persist.tile([128, NT], f32r, tag="yT2")
            wp_t = persist.tile([128, C], f32r, tag="wp")
            bias_t = persist.tile([128, 6], f32, tag="bias")
            mask_t = persist.tile([128, 128], dt.float16, tag="mask")
            ones_t = persist.tile([128, 1], dt.float16, tag="ones")

            nc.sync.dma_start(out=wp_t[:], in_=wp[:])
            nc.sync.dma_start(out=bias_t[:], in_=bias6[:])
            nc.sync.dma_start(out=mask_t[:], in_=maskd[:])

            identf = persist.tile([128, 128], f32, tag="identf")
            make_identity(nc, identf[:])
            ident_t = persist.tile([128, 128], f32r, tag="ident")
            nc.vector.tensor_copy(ident_t[:], identf[:])
            onesf = persist.tile([128, 1], f32, tag="onesf")
            nc.vector.memset(onesf[:], 1.0)
            nc.vector.tensor_copy(ones_t[:], onesf[:])
            zcolf = persist.tile([1, 128], f32, tag="zcolf")
            nc.vector.memset(zcolf[:], 0.0)
            zcol_t = persist.tile([1, 128], f32r, tag="zcol")
            nc.vector.tensor_copy(zcol_t[:], zcolf[:])
            zrowf = persist.tile([1, 512], f32, tag="zrowf")
            nc.vector.memset(zrowf[:], 0.0)
            zrow_t = persist.tile([1, 512], f32r, tag="zrow")
            nc.vector.tensor_copy(zrow_t[:], zrowf[:])

            # ---------------- Phase A: projections ----------------
            with ExitStack() as esa:
                wpool = esa.enter_context(tc.tile_pool(name="wpool", bufs=1))
                apool = esa.enter_context(tc.tile_pool(name="apool", bufs=2))
                vtpool = esa.enter_context(tc.tile_pool(name="vtpool", bufs=1))
                aps = esa.enter_context(tc.tile_pool(name="aps", bufs=2, space="PSUM"))
                apt = esa.enter_context(tc.tile_pool(name="apt", bufs=2, space="PSUM"))

                wq_t = wpool.tile([128, KT_X, CPC], f32r, tag="wq")
                wk_t = wpool.tile([128, KT_X, CPC], f32r, tag="wk")
                wv_t = wpool.tile([128, KT_X, CPC], f32r, tag="wv")
                wcq_t = wpool.tile([128, KT_X, CPC], f32r, tag="wcq")
                wck_t = wpool.tile([128, KT_C, CPC], f32r, tag="wck")
                wcv_t = wpool.tile([128, KT_C, CPC], f32r, tag="wcv")
                for wdram, wtile, ktn in ((wq, wq_t, KT_X), (wk, wk_t, KT_X),
                                          (wv, wv_t, KT_X), (wcq, wcq_t, KT_X),
                                          (wck, wck_t, KT_C), (wcv, wcv_t, KT_C)):
                    nc.sync.dma_start(
                        out=wtile[:],
                        in_=wdram.rearrange("(kt p) d -> p kt d", p=128))

                vT_t = vtpool.tile([128, NT], f32r, tag="vT")
                vcT_t = vtpool.tile([128, NTC], f32r, tag="vcT")

                xT_r = xT.rearrange("(kt p) t -> p kt t", p=128)
                for ch in range(NCH):
                    xblk = apool.tile([128, KT_X, 512], f32r, tag="xblk")
                    nc.sync.dma_start(out=xblk[:], in_=xT_r[:, :, ch * 512:(ch + 1) * 512])
                    for wtile, dst, bcol in ((wq_t, qT_t, 0), (wk_t, kT_t, 1),
                                             (wv_t, vT_t, 2), (wcq_t, qcT_t, 3)):
                        ps = aps.tile([128, 512], f32, tag="aps")
                        for kt in range(KT_X):
                            nc.tensor.matmul(ps[:], wtile[:, kt, :], xblk[:, kt, :],
                                             start=(kt == 0), stop=(kt == KT_X - 1))
                        if zero_bias:
                            nc.scalar.copy(dst[:, ch * 512:(ch + 1) * 512], ps[:])
                        else:
                            nc.vector.tensor_scalar_add(
                                dst[:, ch * 512:(ch + 1) * 512], ps[:],
                                bias_t[:, bcol:bcol + 1])

                cT_r = cT.rearrange("(kt p) t -> p kt t", p=128)
                for ch in range(NCHC):
                    cblk = apool.tile([128, KT_C, 512], f32r, tag="cblk")
                    nc.sync.dma_start(out=cblk[:], in_=cT_r[:, :, ch * 512:(ch + 1) * 512])
                    for wtile, dst, bcol in ((wck_t, kcT_t, 4), (wcv_t, vcT_t, 5)):
                        ps = aps.tile([128, 512], f32, tag="aps")
                        for kt in range(KT_C):
                            nc.tensor.matmul(ps[:], wtile[:, kt, :], cblk[:, kt, :],
                                             start=(kt == 0), stop=(kt == KT_C - 1))
                        if zero_bias:
                            nc.scalar.copy(dst[:, ch * 512:(ch + 1) * 512], ps[:])
                        else:
                            nc.vector.tensor_scalar_add(
                                dst[:, ch * 512:(ch + 1) * 512], ps[:],
                                bias_t[:, bcol:bcol + 1])

                # Phase A2: vT/vcT -> natural token-major layout via PE transpose
                for tt in range(NT // 128):
                    pt = apt.tile([128, 128], f32r, tag="apt")
                    nc.tensor.transpose(pt[:], vT_t[:, tt * 128:(tt + 1) * 128], ident_t[:])
                    nc.vector.tensor_copy(vn_t[:, tt * 128:(tt + 1) * 128], pt[:])
                for tt in range(NTC // 128):
                    pt = apt.tile([128, 128], f32r, tag="apt")
                    nc.tensor.transpose(pt[:], vcT_t[:, tt * 128:(tt + 1) * 128], ident_t[:])
                    nc.vector.tensor_copy(vcn_t[:, tt * 128:(tt + 1) * 128], pt[:])

            # ---------------- Phase B: attention ----------------
            with ExitStack() as esb:
                bpool = esb.enter_context(tc.tile_pool(name="bpool", bufs=2))
                expool = esb.enter_context(tc.tile_pool(name="expool", bufs=4))
                stps = esb.enter_context(tc.tile_pool(name="stps", bufs=3, space="PSUM"))
                yps = esb.enter_context(tc.tile_pool(name="yps", bufs=2, space="PSUM"))
                dnps = esb.enter_context(tc.tile_pool(name="dnps", bufs=1, space="PSUM"))

                for b in range(B):
                    for qc in range(QC_PER_B):
                        qlo = b * T + qc * 512
                        yT = yps.tile([128, 512], f32, tag="yT")
                        yTc = yps.tile([128, 512], f32, tag="yTc")
                        dns = dnps.tile([97, 512], f32, tag="dns")
                        nc.tensor.matmul(yT[:], zcol_t[:], zrow_t[:],
                                         start=True, stop=False)
                        nc.tensor.matmul(dns[:], zcol_t[:, 0:97], zrow_t[:],
                                         start=True, stop=False)
                        nc.tensor.matmul(yTc[:], zcol_t[:], zrow_t[:],
                                         start=True, stop=False)

                        # causal self-attention over k-tiles 0..4*qc+3
                        nkt = 4 * qc + 4
                        for kt in range(nkt):
                            crossing = kt >= 4 * qc
                            off = (kt - 4 * qc) * 128 if crossing else 0
                            klo = b * T + kt * 128
                            for h in range(2):
                                hp = h * 64
                                st = stps.tile([128, 512], f32, tag="st")
                                nc.tensor.matmul(
                                    st[:, off:512],
                                    kT_t[hp:hp + 64, klo:klo + 128],
                                    qT_t[hp:hp + 64, qlo + off:qlo + 512],
                                    start=True, stop=True)
                                ex = expool.tile([128, 512], dt.float16, tag="ex")
                                nc.scalar.activation(ex[:, off:512], st[:, off:512],
                                                     Exp, scale=SCALE)
                                if crossing:
                                    nc.vector.tensor_mul(ex[:, off:off + 128],
                                                         ex[:, off:off + 128],
                                                         mask_t[:])
                                vcol = (b * KT_PER_B + kt) * 128 + hp
                                nc.tensor.matmul(
                                    yT[hp:hp + 64, off:512],
                                    vn_t[:, vcol:vcol + 64],
                                    ex[:, off:512],
                                    tile_position=(0, hp),
                                    start=False, stop=False)
                                nc.tensor.matmul(
                                    dns[h * 32:h * 32 + 1, off:512],
                                    ones_t[:],
                                    ex[:, off:512],
                                    tile_position=(0, h * 32),
                                    start=False, stop=False)

                        # cross-attention (no mask), k-tiles over TC=512
                        for kt in range(KT_C):
                            klo = b * TC + kt * 128
                            for h in range(2):
                                hp = h * 64
                                st = stps.tile([128, 512], f32, tag="st")
                                nc.tensor.matmul(
                                    st[:],
                                    kcT_t[hp:hp + 64, klo:klo + 128],
                                    qcT_t[hp:hp + 64, qlo:qlo + 512],
                                    start=True, stop=True)
                                ex = expool.tile([128, 512], dt.float16, tag="ex")
                                nc.scalar.activation(ex[:], st[:], Exp, scale=SCALE)
                                vcol = (b * KT_C + kt) * 128 + hp
                                nc.tensor.matmul(
                                    yTc[hp:hp + 64, :],
                                    vcn_t[:, vcol:vcol + 64],
                                    ex[:],
                                    tile_position=(0, hp),
                                    start=False, stop=False)
                                nc.tensor.matmul(
                                    dns[64 + h * 32:64 + h * 32 + 1, :],
                                    ones_t[:],
                                    ex[:],
                                    tile_position=(0, 64 + h * 32),
                                    start=False, stop=False)

                        # close the psum accumulation groups (full-tile +0)
                        nc.tensor.matmul(yT[:], zcol_t[:], zrow_t[:],
                                         start=False, stop=True)
                        nc.tensor.matmul(dns[:], zcol_t[:, 0:97], zrow_t[:],
                                         start=False, stop=True)
                        nc.tensor.matmul(yTc[:], zcol_t[:], zrow_t[:],
                                         start=False, stop=True)

                        # normalize: y = yT/dn + yTc/dnc  (per-head denominators)
                        rcp_s = bpool.tile([1, 1024], f32, tag="rcps")
                        nc.vector.reciprocal(rcp_s[:, 0:512], dns[0:1, :])
                        nc.vector.reciprocal(rcp_s[:, 512:1024], dns[32:33, :])
                        rcp_c = bpool.tile([1, 1024], f32, tag="rcpc")
                        nc.vector.reciprocal(rcp_c[:, 0:512], dns[64:65, :])
                        nc.vector.reciprocal(rcp_c[:, 512:1024], dns[96:97, :])
                        bc_s = bpool.tile([128, 1024], f32, tag="bcs")
                        nc.gpsimd.partition_broadcast(bc_s[:], rcp_s[:])
                        bc_c = bpool.tile([128, 1024], f32, tag="bcc")
                        nc.gpsimd.partition_broadcast(bc_c[:], rcp_c[:])
                        y1 = bpool.tile([128, 512], f32, tag="y1")
                        nc.vector.tensor_mul(y1[0:64, :], yT[0:64, :], bc_s[0:64, 0:512])
                        nc.vector.tensor_mul(y1[64:128, :], yT[64:128, :], bc_s[64:128, 512:1024])
                        y2 = bpool.tile([128, 512], f32, tag="y2")
                        nc.vector.tensor_mul(y2[0:64, :], yTc[0:64, :], bc_c[0:64, 0:512])
                        nc.vector.tensor_mul(y2[64:128, :], yTc[64:128, :], bc_c[64:128, 512:1024])
                        nc.vector.tensor_add(yT2_t[:, qlo:qlo + 512], y1[:], y2[:])

            # ---------------- Phase C: output projection ----------------
            with ExitStack() as esc:
                cpool = esc.enter_context(tc.tile_pool(name="cpool", bufs=4))
                cps = esc.enter_context(tc.tile_pool(name="cps", bufs=4, space="PSUM"))
                for tt in range(NT // 128):
                    for co in range(2):
                        po = cps.tile([128, 512], f32, tag="po")
                        nc.tensor.matmul(po[:],
                                         yT2_t[:, tt * 128:(tt + 1) * 128],
                                         wp_t[:, co * 512:(co + 1) * 512],
                                         start=True, stop=True)
                        so = cpool.tile([128, 512], f32, tag="so")
                        nc.scalar.copy(so[:], po[:])
                        nc.sync.dma_start(
                            out=out[tt * 128:(tt + 1) * 128, co * 512:(co + 1) * 512],
                            in_=so[:])
    nc.compile()
    return nc


_NC_CACHE = {}


def _get_nc(zero_bias=False):
    if zero_bias not in _NC_CACHE:
        _NC_CACHE[zero_bias] = _build(zero_bias)
    return _NC_CACHE[zero_bias]


def make_in_maps(x, cross_input, Wk, bk, Wq, bq, Wv, bv, Wck, bck, Wcq, bcq,
                 Wcv, bcv, Wp, bp):
    """Host-side shard + layout prep. Returns per-core input maps."""
    xT = _round_fp32r(np.asarray(x, np.float32).reshape(NT, C).T)
    cT = _round_fp32r(np.asarray(cross_input, np.float32).reshape(NTC, CC).T)
    mask = np.triu(np.ones((128, 128), np.float32)).astype(np.float16)  # 1 iff kk<=qq
    Wq, Wk, Wv = (np.asarray(w, np.float32) for w in (Wq, Wk, Wv))
    Wcq, Wck, Wcv = (np.asarray(w, np.float32) for w in (Wcq, Wck, Wcv))
    Wp = np.asarray(Wp, np.float32)
    in_maps = []
    for c in range(NCORES):
        sl = slice(c * CPC, (c + 1) * CPC)
        bias6 = np.stack([np.asarray(v, np.float32)[sl] for v in
                          (bq, bk, bv, bcq, bck, bcv)], axis=1)
        in_maps.append({
            "xT": xT, "cT": cT,
            "wq": _round_fp32r(Wq[:, sl]), "wk": _round_fp32r(Wk[:, sl]),
            "wv": _round_fp32r(Wv[:, sl]), "wcq": _round_fp32r(Wcq[:, sl]),
            "wck": _round_fp32r(Wck[:, sl]), "wcv": _round_fp32r(Wcv[:, sl]),
            "wp": _round_fp32r(Wp[sl, :]),
            "bias6": np.ascontiguousarray(bias6),
            "mask": mask,
        })
    return in_maps


def kernel(**inputs):
    in_maps = make_in_maps(**inputs)
    zb = all(not np.any(np.asarray(inputs[k])) for k in
             ("bq", "bk", "bv", "bcq", "bck", "bcv"))
    nc = _get_nc(zero_bias=zb)
    res = run_bass_kernel_spmd(nc, in_maps, list(range(NCORES)))
    acc = np.zeros((NT, C), np.float64)
    for c in range(NCORES):
        acc += res.results[c]["out"]
    acc += np.asarray(inputs["bp"], np.float32)
    return acc.reshape(B, T, C).astype(np.float32)


if __name__ == "__main__":
    nc = _get_nc()
    print("build + compile OK")


# revision 11
# speedup vs baseline: 1.0959x; 1.0959x over previous
"""Causal self-attention + cross-attention Trainium2 kernel (8 NeuronCores).

Sharding: head-parallel. 16 heads x 2 batches = 32 (b,h) pairs; core c owns
heads {2c, 2c+1} for both batches (its 128 channels of C=1024). Projections
are column-sliced per core; attention runs fully local per head; the output
projection is row-sliced and the 8 partial [B*T, C] outputs are summed on
the host (no device collectives).

Numerics: float32r matmuls for projections/scores/output (fp32 data rounded
to 11-bit mantissa; products are exact in fp32, PSUM accumulates fp32);
fp16 for the probability side (exp output, V, masks) which unlocks PE
column-tiling and keeps ~5e-4 accuracy. Softmax without max-subtraction
(scores are bounded ~|8| for this problem's distributions), exp on ScalarE
with the 1/sqrt(D) scale folded in, scores computed transposed (ST[k,q]) so
no probability transpose is needed before AV.

Phase B is software-pipelined: score matmuls are issued LOOKAHEAD steps
ahead of the matching AV/denominator matmuls so the (FIFO) PE queue never
stalls waiting for ScalarE's exp.
"""
import sys

sys.path.insert(0, "/opt/trn_rl_repo")

import numpy as np

import concourse.bass as bass
import concourse.tile as tile
from concourse import bacc, mybir
from concourse.bass_utils import run_bass_kernel_spmd
from concourse.masks import make_identity

dt = mybir.dt

B, T, TC, C, CC, H, D = 2, 2048, 512, 1024, 512, 16, 64
NCORES = 8
CPC = 128          # channels per core = 2 heads * 64
NT = B * T         # 4096 tokens (batch-major)
NTC = B * TC       # 1024 cross tokens
KT_X = C // 128    # 8 contraction tiles over C
KT_C = CC // 128   # 4 contraction tiles over CC
NCH = NT // 512    # 8 token chunks
NCHC = NTC // 512  # 2 cross token chunks
QC_PER_B = T // 512  # 4 q-chunks per batch
KT_PER_B = T // 128  # 16 k-tiles per batch
LOOKAHEAD = 3      # score matmuls issued ahead of their AV in the PE queue


def _round_fp32r(x):
    """Round fp32 array to fp32r (11-bit mantissa, RNE) on the host."""
    u = np.ascontiguousarray(x, np.float32).view(np.uint32).copy()
    u += 0x7FF + ((u >> 12) & 1)
    u &= 0xFFFFF000
    return u.view(np.float32)


def _build(zero_bias=False):
    f32, f32r, f16 = dt.float32, dt.float32r, dt.float16
    nc = bacc.Bacc("TRN2", target_bir_lowering=False, debug=False,
                   enable_asserts=True, num_devices=NCORES)

    xT = nc.dram_tensor("xT", [C, NT], f32r, kind="ExternalInput").ap()
    cT = nc.dram_tensor("cT", [CC, NTC], f32r, kind="ExternalInput").ap()
    wq = nc.dram_tensor("wq", [C, CPC], f32r, kind="ExternalInput").ap()
    wk = nc.dram_tensor("wk", [C, CPC], f32r, kind="ExternalInput").ap()
    wv = nc.dram_tensor("wv", [C, CPC], f32r, kind="ExternalInput").ap()
    wcq = nc.dram_tensor("wcq", [C, CPC], f32r, kind="ExternalInput").ap()
    wck = nc.dram_tensor("wck", [CC, CPC], f32r, kind="ExternalInput").ap()
    wcv = nc.dram_tensor("wcv", [CC, CPC], f32r, kind="ExternalInput").ap()
    wp = nc.dram_tensor("wp", [CPC, C], f32r, kind="ExternalInput").ap()
    bias6 = nc.dram_tensor("bias6", [CPC, 6], f32, kind="ExternalInput").ap()
    maskd = nc.dram_tensor("mask", [128, 128], f16, kind="ExternalInput").ap()
    out = nc.dram_tensor("out", [NT, C], f32, kind="ExternalOutput").ap()

    Exp = mybir.ActivationFunctionType.Exp
    SCALE = 0.125  # 1/sqrt(D)

    with tile.TileContext(nc) as tc:
        from contextlib import ExitStack
        with ExitStack() as es:
            persist = es.enter_context(tc.tile_pool(name="persist", bufs=1))
            qT_t = persist.tile([128, NT], f32r, tag="qT")
            kT_t = persist.tile([128, NT], f32r, tag="kT")
            qcT_t = persist.tile([128, NT], f32r, tag="qcT")
            kcT_t = persist.tile([128, NTC], f32r, tag="kcT")
            vn_t = persist.tile([128, NT], f16, tag="vn")     # v natural, col=tile*128+ch
            vcn_t = persist.tile([128, NTC], f16, tag="vcn")
            yT2_t = persist.tile([128, NT], f32r, tag="yT2")
            wp_t = persist.tile([128, C], f32r, tag="wp")
            bias_t = persist.tile([128, 6], f32, tag="bias")
            mask_t = persist.tile([128, 128], f16, tag="mask")
            ones_t = persist.tile([128, 1], f16, tag="ones")

            nc.sync.dma_start(out=wp_t[:], in_=wp[:])
            nc.sync.dma_start(out=bias_t[:], in_=bias6[:])
            nc.sync.dma_start(out=mask_t[:], in_=maskd[:])

            identf = persist.tile([128, 128], f32, tag="identf")
            make_identity(nc, identf[:])
            ident_t = persist.tile([128, 128], f32r, tag="ident")
            nc.vector.tensor_copy(ident_t[:], identf[:])
            onesf = persist.tile([128, 1], f32, tag="onesf")
            nc.vector.memset(onesf[:], 1.0)
            nc.vector.tensor_copy(ones_t[:], onesf[:])
            zcolf = persist.tile([1, 128], f32, tag="zcolf")
            nc.vector.memset(zcolf[:], 0.0)
            zcol_t = persist.tile([1, 128], f32r, tag="zcol")
            nc.vector.tensor_copy(zcol_t[:], zcolf[:])
            zrowf = persist.tile([1, 512], f32, tag="zrowf")
            nc.vector.memset(zrowf[:], 0.0)
            zrow_t = persist.tile([1, 512], f32r, tag="zrow")
            nc.vector.tensor_copy(zrow_t[:], zrowf[:])

            # ---------------- Phase A: projections ----------------
            with ExitStack() as esa:
                wpool = esa.enter_context(tc.tile_pool(name="wpool", bufs=1))
                apool = esa.enter_context(tc.tile_pool(name="apool", bufs=2))
                vtpool = esa.enter_context(tc.tile_pool(name="vtpool", bufs=1))
                aps = esa.enter_context(tc.tile_pool(name="aps", bufs=3, space="PSUM"))
                apt = esa.enter_context(tc.tile_pool(name="apt", bufs=3, space="PSUM"))

                wq_t = wpool.tile([128, KT_X, CPC], f32r, tag="wq")
                wk_t = wpool.tile([128, KT_X, CPC], f32r, tag="wk")
                wv_t = wpool.tile([128, KT_X, CPC], f32r, tag="wv")
                wcq_t = wpool.tile([128, KT_X, CPC], f32r, tag="wcq")
                wck_t = wpool.tile([128, KT_C, CPC], f32r, tag="wck")
                wcv_t = wpool.tile([128, KT_C, CPC], f32r, tag="wcv")
                for wdram, wtile in ((wq, wq_t), (wk, wk_t), (wv, wv_t),
                                     (wcq, wcq_t), (wck, wck_t), (wcv, wcv_t)):
                    nc.sync.dma_start(
                        out=wtile[:],
                        in_=wdram.rearrange("(kt p) d -> p kt d", p=128))

                vT_t = vtpool.tile([128, NT], f32r, tag="vT")
                vcT_t = vtpool.tile([128, NTC], f32r, tag="vcT")

                def psum_evict(dst_slice, ps, bcol):
                    if zero_bias:
                        nc.scalar.copy(dst_slice, ps[:])
                    else:
                        nc.vector.tensor_scalar_add(dst_slice, ps[:],
                                                    bias_t[:, bcol:bcol + 1])

                xT_r = xT.rearrange("(kt p) t -> p kt t", p=128)
                for ch in range(NCH):
                    xblk = apool.tile([128, KT_X, 512], f32r, tag="xblk")
                    nc.sync.dma_start(out=xblk[:], in_=xT_r[:, :, ch * 512:(ch + 1) * 512])
                    for wtile, dst, bcol in ((wq_t, qT_t, 0), (wk_t, kT_t, 1),
                                             (wv_t, vT_t, 2), (wcq_t, qcT_t, 3)):
                        ps = aps.tile([128, 512], f32, tag="aps")
                        for kt in range(KT_X):
                            nc.tensor.matmul(ps[:], wtile[:, kt, :], xblk[:, kt, :],
                                             start=(kt == 0), stop=(kt == KT_X - 1))
                        psum_evict(dst[:, ch * 512:(ch + 1) * 512], ps, bcol)

                cT_r = cT.rearrange("(kt p) t -> p kt t", p=128)
                for ch in range(NCHC):
                    cblk = apool.tile([128, KT_C, 512], f32r, tag="cblk")
                    nc.sync.dma_start(out=cblk[:], in_=cT_r[:, :, ch * 512:(ch + 1) * 512])
                    for wtile, dst, bcol in ((wck_t, kcT_t, 4), (wcv_t, vcT_t, 5)):
                        ps = aps.tile([128, 512], f32, tag="aps")
                        for kt in range(KT_C):
                            nc.tensor.matmul(ps[:], wtile[:, kt, :], cblk[:, kt, :],
                                             start=(kt == 0), stop=(kt == KT_C - 1))
                        psum_evict(dst[:, ch * 512:(ch + 1) * 512], ps, bcol)

                # Phase A2: vT/vcT -> natural token-major fp16 layout via PE transpose
                for tt in range(NT // 128):
                    pt = apt.tile([128, 128], f32r, tag="apt")
                    nc.tensor.transpose(pt[:], vT_t[:, tt * 128:(tt + 1) * 128], ident_t[:])
                    nc.vector.tensor_copy(vn_t[:, tt * 128:(tt + 1) * 128], pt[:])
                for tt in range(NTC // 128):
                    pt = apt.tile([128, 128], f32r, tag="apt")
                    nc.tensor.transpose(pt[:], vcT_t[:, tt * 128:(tt + 1) * 128], ident_t[:])
                    nc.vector.tensor_copy(vcn_t[:, tt * 128:(tt + 1) * 128], pt[:])

            # ---------------- Phase B: attention ----------------
            with ExitStack() as esb:
                bpool = esb.enter_context(tc.tile_pool(name="bpool", bufs=2))
                ypool = esb.enter_context(tc.tile_pool(name="ypool", bufs=4))
                expool = esb.enter_context(tc.tile_pool(name="expool", bufs=6))
                stps = esb.enter_context(tc.tile_pool(name="stps", bufs=4, space="PSUM"))
                yps = esb.enter_context(tc.tile_pool(name="yps", bufs=2, space="PSUM"))
                dnps = esb.enter_context(tc.tile_pool(name="dnps", bufs=2, space="PSUM"))

                def attn_part(b, qc, qlo, is_self):
                    """One softmax-attention accumulation (self or cross) for a
                    512-wide q chunk of batch b; returns normalized [128,512]."""
                    nkt = (4 * qc + 4) if is_self else KT_C
                    yTp = yps.tile([128, 512], f32, tag="yT")
                    dnp = dnps.tile([33, 512], f32, tag="dns")
                    nc.tensor.matmul(yTp[:], zcol_t[:], zrow_t[:],
                                     start=True, stop=False)
                    nc.tensor.matmul(dnp[:], zcol_t[:, 0:33], zrow_t[:],
                                     start=True, stop=False)

                    pend = []

                    def flush_one():
                        ex, off, vsrc, vcol, hp = pend.pop(0)
                        nc.tensor.matmul(
                            yTp[hp:hp + 64, off:512],
                            vsrc[:, vcol:vcol + 64],
                            ex[:, off:512],
                            tile_position=(0, hp),
                            start=False, stop=False)
                        nc.tensor.matmul(
                            dnp[(hp // 2):(hp // 2) + 1, off:512],
                            ones_t[:],
                            ex[:, off:512],
                            tile_position=(0, hp // 2),
                            start=False, stop=False)

                    for kt in range(nkt):
                        for h in range(2):
                            hp = h * 64
                            if is_self:
                                crossing = kt >= 4 * qc
                                off = (kt - 4 * qc) * 128 if crossing else 0
                                klo = b * T + kt * 128
                                ksrc, qsrc, vsrc = kT_t, qT_t, vn_t
                                vcol = (b * KT_PER_B + kt) * 128 + hp
                            else:
                                crossing, off = False, 0
                                klo = b * TC + kt * 128
                                ksrc, qsrc, vsrc = kcT_t, qcT_t, vcn_t
                                vcol = (b * KT_C + kt) * 128 + hp
                            st = stps.tile([128, 512], f32, tag="st")
                            nc.tensor.matmul(
                                st[:, off:512],
                                ksrc[hp:hp + 64, klo:klo + 128],
                                qsrc[hp:hp + 64, qlo + off:qlo + 512],
                                start=True, stop=True)
                            ex = expool.tile([128, 512], f16, tag="ex")
                            nc.scalar.activation(ex[:, off:512], st[:, off:512],
                                                 Exp, scale=SCALE)
                            if crossing:
                                nc.vector.tensor_mul(ex[:, off:off + 128],
                                                     ex[:, off:off + 128],
                                                     mask_t[:])
                            pend.append((ex, off, vsrc, vcol, hp))
                            if len(pend) > LOOKAHEAD:
                                flush_one()
                    while pend:
                        flush_one()

                    # close the accumulation groups (full-tile +0)
                    nc.tensor.matmul(yTp[:], zcol_t[:], zrow_t[:],
                                     start=False, stop=True)
                    nc.tensor.matmul(dnp[:], zcol_t[:, 0:33], zrow_t[:],
                                     start=False, stop=True)

                    # normalize: per-head reciprocal, broadcast, multiply
                    rcp = bpool.tile([1, 1024], f32, tag="rcp")
                    nc.vector.reciprocal(rcp[:, 0:512], dnp[0:1, :])
                    nc.vector.reciprocal(rcp[:, 512:1024], dnp[32:33, :])
                    bc = bpool.tile([128, 1024], f32, tag="bc")
                    nc.gpsimd.partition_broadcast(bc[:], rcp[:])
                    yn = ypool.tile([128, 512], f32, tag="yn")
                    nc.vector.tensor_mul(yn[0:64, :], yTp[0:64, :], bc[0:64, 0:512])
                    nc.vector.tensor_mul(yn[64:128, :], yTp[64:128, :],
                                         bc[64:128, 512:1024])
                    return yn

                for b in range(B):
                    for qc in range(QC_PER_B):
                        qlo = b * T + qc * 512
                        y2 = attn_part(b, qc, qlo, is_self=False)
                        y1 = attn_part(b, qc, qlo, is_self=True)
                        nc.vector.tensor_add(yT2_t[:, qlo:qlo + 512], y1[:], y2[:])

            # ---------------- Phase C: output projection ----------------
            with ExitStack() as esc:
                cpool = esc.enter_context(tc.tile_pool(name="cpool", bufs=4))
                cps = esc.enter_context(tc.tile_pool(name="cps", bufs=4, space="PSUM"))
                for tt in range(NT // 128):
                    for co in range(2):
                        po = cps.tile([128, 512], f32, tag="po")
                        nc.tensor.matmul(po[:],
                                         yT2_t[:, tt * 128:(tt + 1) * 128],
                                         wp_t[:, co * 512:(co + 1) * 512],
                                         start=True, stop=True)
                        so = cpool.tile([128, 512], f32, tag="so")
                        nc.scalar.copy(so[:], po[:])
                        nc.sync.dma_start(
                            out=out[tt * 128:(tt + 1) * 128, co * 512:(co + 1) * 512],
                            in_=so[:])
    nc.compile()
    return nc


_NC_CACHE = {}


def _get_nc(zero_bias=False):
    if zero_bias not in _NC_CACHE:
        _NC_CACHE[zero_bias] = _build(zero_bias)
    return _NC_CACHE[zero_bias]


def make_in_maps(x, cross_input, Wk, bk, Wq, bq, Wv, bv, Wck, bck, Wcq, bcq,
                 Wcv, bcv, Wp, bp):
    """Host-side shard + layout prep. Returns per-core input maps."""
    xT = _round_fp32r(np.asarray(x, np.float32).reshape(NT, C).T)
    cT = _round_fp32r(np.asarray(cross_input, np.float32).reshape(NTC, CC).T)
    mask = np.triu(np.ones((128, 128), np.float32)).astype(np.float16)  # 1 iff kk<=qq
    Wq, Wk, Wv = (np.asarray(w, np.float32) for w in (Wq, Wk, Wv))
    Wcq, Wck, Wcv = (np.asarray(w, np.float32) for w in (Wcq, Wck, Wcv))
    Wp = np.asarray(Wp, np.float32)
    in_maps = []
    for c in range(NCORES):
        sl = slice(c * CPC, (c + 1) * CPC)
        bias6 = np.stack([np.asarray(v, np.float32)[sl] for v in
                          (bq, bk, bv, bcq, bck, bcv)], axis=1)
        in_maps.append({
            "xT": xT, "cT": cT,
            "wq": _round_fp32r(Wq[:, sl]), "wk": _round_fp32r(Wk[:, sl]),
            "wv": _round_fp32r(Wv[:, sl]), "wcq": _round_fp32r(Wcq[:, sl]),
            "wck": _round_fp32r(Wck[:, sl]), "wcv": _round_fp32r(Wcv[:, sl]),
            "wp": _round_fp32r(Wp[sl, :]),
            "bias6": np.ascontiguousarray(bias6),
            "mask": mask,
        })
    return in_maps


def kernel(**inputs):
    in_maps = make_in_maps(**inputs)
    zb = all(not np.any(np.asarray(inputs[k])) for k in
             ("bq", "bk", "bv", "bcq", "bck", "bcv"))
    nc = _get_nc(zero_bias=zb)
    res = run_bass_kernel_spmd(nc, in_maps, list(range(NCORES)))
    acc = np.zeros((NT, C), np.float64)
    for c in range(NCORES):
        acc += res.results[c]["out"]
    acc += np.asarray(inputs["bp"], np.float32)
    return acc.reshape(B, T, C).astype(np.float32)


if __name__ == "__main__":
    nc = _get_nc()
    print("build + compile OK")
